# revision 8
# baseline (speedup 1.0000x reference)
"""Trainium2 Bass kernel for nn_CriticHead (critic head over C*t tasks).

Contract: kernel(**inputs) takes the FULL unsharded inputs (as produced by
setup_inputs()) and returns the FULL [1, T] float32 output.  Internally the
work is sharded data-parallel over the leading cluster axis across 8
NeuronCores; the tiny MLP weights are replicated.

Math (per task j, verified against the reference):
    me_j   = mean(enode[j,:])                       # since y41 = y2 * me
    sc_j   = sum(ccl[j,:]) * sum(cnd[j,:])          # since y42 = y2 * sc
    u_j    = [bb_j (768) ; outer3(res_j, fr_j, estep_j) (150)]   # 918 (permuted)
    y2_j   = relu(W1p.T u_j + b1)                   # 128
    a3     = me*(y2@W3)+b3 ; a5 = sc*(y2@W5)+b5     # sigmoid-gated pair
    a4     = me*(y2@W4)+b4 ; a6 = sc*(y2@W6)+b6     # linear pair
    p      = sig(a3)*sig(a5)
    y      = FAILC + p*((a4+a6) - FAILC)

Precision: bb and the bb-part of W1 are bf16 hi+lo pairs; the matmul
accumulates uh@wh + uh@wl + ul@wh (ul@wl negligible).  outer3 features and
their W1 rows are single bf16 (measured ~2.5e-3 rel vs the 2e-2 gate).

Perf notes (from trace analysis):
  - each HWDGE dma_start costs ~600ns of serialized DIRECT2D descriptor
    generation on its sequencer -> merge everything into 9 starts.
  - the PE runs at 1.2GHz (HAM cold) until it has been busy ~3.4us ->
    warm-up matmuls bridge the first DMA wait so the real stream is gapless
    and mostly warm (2.4GHz).
  - DMA floor for the ~2.15MB/core is ~6us; the k-chunk packs are ordered
    so each accumulation matmul's operand arrives just in time.
"""

import sys

if "/opt/trn_rl_repo" not in sys.path:
    sys.path.insert(0, "/opt/trn_rl_repo")

from contextlib import ExitStack

import numpy as np

import concourse.bass as bass
import concourse.mybir as mybir
import concourse.tile as tile
from concourse.bass_utils import run_bass_kernel_spmd

# Problem constants (hardcoded per the harness contract).
NCORES = 8
C, TASKS = 64, 64
T = C * TASKS                 # 4096
TC = T // NCORES              # 512 tasks per core
D_BB = 768
N_OUT = 150                   # 5*5*6 outer-product features
D_H = 128
E_N = 64
C_C, C_N = 4, 32
FAILC = -100.0
NTILE = TC // 128             # 4 task tiles of 128 per core
NBB = D_BB // 128             # 6 bb k-chunks
N_WARM = 6                    # PE warm-up matmuls while first DMAs land

F32 = mybir.dt.float32
BF16 = mybir.dt.bfloat16


def _build_module():
    nc = bass.Bass()

    # w1hi cols: [0:768) bb-hi w1 chunks, [768:896) o3a w1 rows
    w1hi = nc.declare_dram_parameter("w1hi", [128, 896], BF16, isOutput=False)
    w1lo = nc.declare_dram_parameter("w1lo", [128, 768], BF16, isOutput=False)
    # u k-chunk packs, in arrival order: o3a, uh0..uh5, ul0..ul5
    upkA = nc.declare_dram_parameter("upkA", [128, 3, TC], BF16, isOutput=False)
    upkB = nc.declare_dram_parameter("upkB", [128, 3, TC], BF16, isOutput=False)
    upkC = nc.declare_dram_parameter("upkC", [128, 3, TC], BF16, isOutput=False)
    upkD = nc.declare_dram_parameter("upkD", [128, 4, TC], BF16, isOutput=False)
    # o3t cols: [0:128) W1 rows for o3b, [128:640) o3b features, [640:768) b1
    o3t = nc.declare_dram_parameter("o3t", [22, 768], BF16, isOutput=False)
    mesc4 = nc.declare_dram_parameter("mesc4", [128, NTILE, 4], F32, isOutput=False)
    # misc9 cols: [0:4) wh (W3,W5,W4,W6), [4:8) bh' (b3,b5,b4,b6-FAILC), [8:9) b1
    misc9 = nc.declare_dram_parameter("misc9", [128, 9], F32, isOutput=False)
    out = nc.declare_dram_parameter("out", [128, NTILE], F32, isOutput=True)

    with tile.TileContext(nc) as tc, ExitStack() as ctx:
        pool = ctx.enter_context(tc.tile_pool(name="main", bufs=1))
        psum = ctx.enter_context(tc.tile_pool(name="psum", bufs=1, space="PSUM"))

        # PE warm-up tile (HAM): dummy matmuls keep the PE clock at 2.4GHz.
        wz = pool.tile([128, TC], BF16, tag="wz")
        nc.vector.memset(wz, 0.0)
        pwz = psum.tile([128, TC], F32, tag="pwz")

        def warm(n):
            for _ in range(n):
                nc.tensor.matmul(pwz, lhsT=wz[:, 0:D_H], rhs=wz, start=True, stop=True)

        # ---- big loads on the sync HWDGE ring, in consumption order -------
        w1h_s = pool.tile([128, 896], BF16, tag="w1h")
        nc.sync.dma_start(out=w1h_s, in_=w1hi[:, :])
        ups = []
        for name, par, nslot in (("A", upkA, 3), ("B", upkB, 3),
                                 ("C", upkC, 3), ("D", upkD, 4)):
            t = pool.tile([128, nslot, TC], BF16, tag=f"up{name}")
            nc.sync.dma_start(out=t, in_=par[:, :, :])
            ups.append(t)
        upA, upB, upC, upD = ups

        # ---- small loads: gpsimd SWDGE (fast start) + scalar HWDGE ring ---
        o3t_s = pool.tile([22, 768], BF16, tag="o3t")
        nc.gpsimd.dma_start(out=o3t_s, in_=o3t[:, :])
        mesc_s = pool.tile([128, NTILE, 4], F32, tag="mesc")
        nc.gpsimd.dma_start(out=mesc_s, in_=mesc4[:, :, :])
        w1l_s = pool.tile([128, 768], BF16, tag="w1l")
        nc.scalar.dma_start(out=w1l_s, in_=w1lo[:, :])
        misc_s = pool.tile([128, 9], F32, tag="misc")
        nc.scalar.dma_start(out=misc_s, in_=misc9[:, :])

        # Preload the sigmoid ACT table (after the scalar DMAs so they are
        # not blocked behind the ~1.3us table load).
        sgw = pool.tile([32, 1], F32, tag="sgw")
        nc.vector.memset(sgw, 0.0)
        ones1 = pool.tile([1, TC], BF16, tag="ones1")
        nc.vector.memset(ones1, 1.0)
        nc.scalar.activation(sgw, sgw, mybir.ActivationFunctionType.Sigmoid)

        # ---- main matmul: psumY = W1h.T uh + W1l.T uh + W1h.T ul ----------
        psumY = psum.tile([128, TC], F32, tag="psumY")
        n_mm = 3 * NBB + 3
        pos = 0

        def mm(lhsT, rhs):
            nonlocal pos
            nc.tensor.matmul(
                psumY, lhsT=lhsT, rhs=rhs,
                start=(pos == 0), stop=(pos == n_mm - 1))
            pos += 1

        def w1h(j):        # bb-hi chunk j (0..5)
            return w1h_s[:, 128 * j : 128 * (j + 1)]

        def w1l(j):        # bb-lo chunk j (0..5)
            return w1l_s[:, 128 * j : 128 * (j + 1)]

        # chunk layout: upA = (o3a, uh0, uh1), upB = (uh2, uh3, uh4),
        # upC = (uh5, ul0, ul1), upD = (ul2..ul5)
        uh_sl = [upA[:, 1, :], upA[:, 2, :], upB[:, 0, :],
                 upB[:, 1, :], upB[:, 2, :], upC[:, 0, :]]
        ul_sl = [upC[:, 1, :], upC[:, 2, :], upD[:, 0, :],
                 upD[:, 1, :], upD[:, 2, :], upD[:, 3, :]]

        warm(2)
        mm(o3t_s[:, 0:128], o3t_s[:, 128:640])  # o3b (k=22), gpsimd ring, early
        mm(o3t_s[0:1, 640:768], ones1)          # b1 via k=1 ones-row matmul
        for _ in range(2):                      # fillers: ready with o3t, keep
            nc.tensor.matmul(                   # the PE busy until upA lands
                pwz, lhsT=o3t_s[:, 0:128], rhs=o3t_s[:, 128:640],
                start=True, stop=True)
        mm(w1h_s[:, 768:896], upA[:, 0, :])     # o3a
        mm(w1h(0), uh_sl[0])
        mm(w1h(1), uh_sl[1])
        mm(w1l(0), uh_sl[0])                    # wl fillers: no new DMA needed
        mm(w1l(1), uh_sl[1])
        mm(w1h(2), uh_sl[2])
        mm(w1h(3), uh_sl[3])
        mm(w1h(4), uh_sl[4])
        mm(w1l(2), uh_sl[2])
        mm(w1l(3), uh_sl[3])
        mm(w1l(4), uh_sl[4])
        mm(w1h(5), uh_sl[5])
        mm(w1l(5), uh_sl[5])
        for j in range(NBB):
            mm(w1h(j), ul_sl[j])                # ul@wh: paced by the ul DMAs

        # ---- relu in halves on two engines (ACT + DVE run in parallel);
        # b1 is already accumulated in psum via the ones-row matmul.
        y2T = pool.tile([128, TC], F32, tag="y2T")
        nc.scalar.activation(
            y2T[:, 0:256], psumY[:, 0:256],
            mybir.ActivationFunctionType.Relu)
        nc.vector.tensor_scalar_max(y2T[:, 256:512], psumY[:, 256:512], 0.0)

        # ---- heads, task-major: one 128-task tile at a time --------------
        # cols of psumS[:, i, :]: d3, d5, d4, d6  (W3, W5, W4, W6 order)
        psumS = psum.tile([128, NTILE, 4], F32, tag="psumS")
        for i in range(NTILE):
            nc.tensor.matmul(
                psumS[:, i, :],
                lhsT=y2T[:, 128 * i : 128 * (i + 1)],
                rhs=misc_s[:, 0:4],
                start=True,
                stop=True,
            )

        # ---- combine ------------------------------------------------------
        # bh' folds -FAILC into the b6 column, so y6s = a4 + a6 - FAILC and
        # out = pv*y6s + FAILC.
        av = pool.tile([128, NTILE, 4], F32, tag="av")
        nc.vector.tensor_mul(av, psumS, mesc_s)
        nc.vector.tensor_add(
            av, av, misc_s[:, 4:8].unsqueeze(1).broadcast_to([128, NTILE, 4])
        )
        y6s = pool.tile([128, NTILE, 1], F32, tag="y6s")
        nc.vector.tensor_add(y6s, av[:, :, 2:3], av[:, :, 3:4])

        sg = pool.tile([128, NTILE, 2], F32, tag="sg")
        nc.scalar.activation(sg, av[:, :, 0:2], mybir.ActivationFunctionType.Sigmoid)

        pv = pool.tile([128, NTILE, 1], F32, tag="pv")
        nc.vector.tensor_mul(pv, sg[:, :, 0:1], sg[:, :, 1:2])
        om = pool.tile([128, NTILE, 1], F32, tag="om")
        nc.vector.tensor_mul(om, pv, y6s)
        outv = pool.tile([128, NTILE, 1], F32, tag="outv")
        nc.vector.tensor_scalar_add(outv, om, FAILC)

        nc.sync.dma_start(out=out[:, :], in_=outv[:, :, 0])

    return _split_sync_waits(nc)


def _split_sync_waits(nc, max_waits=1):
    """This container's walrus rejects >1 sem-wait per instruction
    ("Too many sync wait commands"); hoist extras onto same-engine NOPs."""
    nid = 0
    for f in nc.m.functions:
        for bb in f.blocks:
            new = []
            for inst in bb.instructions:
                si = inst.sync_info
                if si is None:
                    new.append(inst)
                    continue
                waits = list(si.on_wait or [])
                if len(waits) > max_waits:
                    for w in waits[:-max_waits]:
                        nop = mybir.InstNoOp(name=f"WSPL-{nid}", ins=[], outs=[])
                        nid += 1
                        nop.engine = inst.engine
                        nop.sync_info = mybir.SyncInfo(on_wait=[w], on_update=[])
                        new.append(nop)
                    inst.sync_info = mybir.SyncInfo(
                        on_wait=waits[-max_waits:], on_update=list(si.on_update or [])
                    )
                new.append(inst)
            bb.instructions = new
    return nc


_CACHED_NC = None


def _get_nc():
    global _CACHED_NC
    if _CACHED_NC is None:
        _CACHED_NC = _build_module()
    return _CACHED_NC


def _make_in_maps(inputs: dict) -> list[dict[str, np.ndarray]]:
    f32 = np.float32
    bf16 = np.dtype("bfloat16")

    bb = np.asarray(inputs["backbone_y"], f32).reshape(T, D_BB)
    res = np.asarray(inputs["y_res"], f32).reshape(T, 5)
    fr = np.asarray(inputs["y_fr"], f32).reshape(T, 5)
    estep = np.asarray(inputs["y_estep"], f32).reshape(T, 6)
    enode = np.asarray(inputs["y_enode"], f32).reshape(T, E_N)
    ccl = np.asarray(inputs["y_ccluster"], f32).reshape(T, C_C)
    cnd = np.asarray(inputs["y_cnode"], f32).reshape(T, C_N)

    # outer3 features [T, 150] and per-task scalars (host precompute)
    o3 = np.einsum("tn,tm,to->tnmo", res, fr, estep).reshape(T, N_OUT)
    me = enode.mean(axis=1).astype(f32)
    sc = (ccl.sum(axis=1) * cnd.sum(axis=1)).astype(f32)

    # W1 packed with permuted rows: [bb (768) ; outer3 (150)]
    w1 = np.ascontiguousarray(np.asarray(inputs["W1"], f32))
    w1a = w1[0:N_OUT]        # outer3 rows
    w1b = w1[N_OUT:]         # bb rows [768, 128]
    w1bh = w1b.astype(bf16)
    w1bl = (w1b - w1bh.astype(f32)).astype(bf16)
    w1hi_c = np.ascontiguousarray(
        np.concatenate(
            [
                w1bh.reshape(NBB, 128, D_H).transpose(1, 0, 2).reshape(128, NBB * D_H),
                w1a[0:128].astype(bf16),
            ],
            axis=1,
        )
    )  # [128, 896]
    w1lo_c = np.ascontiguousarray(
        w1bl.reshape(NBB, 128, D_H).transpose(1, 0, 2).reshape(128, NBB * D_H)
    )  # [128, 768]
    b1_col = np.asarray(inputs["b1"], f32).reshape(D_H, 1)

    w3 = np.asarray(inputs["W3"], f32).reshape(D_H, 1)
    w4 = np.asarray(inputs["W4"], f32).reshape(D_H, 1)
    w5 = np.asarray(inputs["W5"], f32).reshape(D_H, 1)
    w6 = np.asarray(inputs["W6"], f32).reshape(D_H, 1)
    bh_row = np.array(
        [
            float(np.asarray(inputs["b3"]).reshape(-1)[0]),
            float(np.asarray(inputs["b5"]).reshape(-1)[0]),
            float(np.asarray(inputs["b4"]).reshape(-1)[0]),
            float(np.asarray(inputs["b6"]).reshape(-1)[0]) - FAILC,
        ],
        f32,
    )
    # [128, 9]: wh | bh' | b1
    misc9_c = np.ascontiguousarray(
        np.concatenate(
            [
                np.concatenate([w3, w5, w4, w6], axis=1),
                np.broadcast_to(bh_row, (128, 4)),
                b1_col,
            ],
            axis=1,
        )
    )

    in_maps = []
    for c in range(NCORES):
        sl = slice(c * TC, (c + 1) * TC)
        bbT = bb[sl].T                       # [768, TC]
        uh_c = bbT.astype(bf16)              # C-contiguous, [6*128, TC]
        ul_c = (bbT - uh_c.astype(f32)).astype(bf16)
        o3T = o3[sl].T.astype(bf16)          # [150, TC]
        # packs in arrival order: o3a, uh0..5, ul0..5
        chunks = [o3T[0:128]] + [uh_c[128 * j : 128 * (j + 1)] for j in range(NBB)] \
            + [ul_c[128 * j : 128 * (j + 1)] for j in range(NBB)]
        st = np.stack(chunks, axis=1)        # [128, 13, TC]
        o3t_c = np.ascontiguousarray(
            np.concatenate(
                [
                    w1a[128:N_OUT].astype(bf16),
                    o3T[128:N_OUT],
                    np.broadcast_to(b1_col.reshape(1, D_H), (22, D_H)).astype(bf16),
                ],
                axis=1,
            )
        )  # [22, 768]
        mesc_c = np.ascontiguousarray(
            np.stack(
                [me[sl].reshape(NTILE, 128).T, sc[sl].reshape(NTILE, 128).T] * 2,
                axis=-1,
            )
        )  # [128, NTILE, 4] = me, sc, me, sc
        in_maps.append(
            {
                "w1hi": w1hi_c,
                "w1lo": w1lo_c,
                "upkA": np.ascontiguousarray(st[:, 0:3]),
                "upkB": np.ascontiguousarray(st[:, 3:6]),
                "upkC": np.ascontiguousarray(st[:, 6:9]),
                "upkD": np.ascontiguousarray(st[:, 9:13]),
                "o3t": o3t_c,
                "mesc4": mesc_c,
                "misc9": misc9_c,
            }
        )
    return in_maps


def _assemble(results: list[dict[str, np.ndarray]]) -> np.ndarray:
    parts = [np.asarray(results[c]["out"]).T.reshape(-1) for c in range(NCORES)]
    return np.concatenate(parts)[None, :].astype(np.float32)


def _run(inputs: dict, trace: bool = False):
    nc = _get_nc()
    in_maps = _make_in_maps(inputs)
    kres = run_bass_kernel_spmd(
        nc, in_maps, core_ids=list(range(NCORES)), trace=trace
    )
    return _assemble(kres.results), kres


def kernel(**inputs) -> np.ndarray:
    out, _ = _run(inputs)
    return out


# revision 9
# speedup vs baseline: 1.0239x; 1.0239x over previous
"""Trainium2 Bass kernel for nn_CriticHead (critic head over C*t tasks).

Contract: kernel(**inputs) takes the FULL unsharded inputs (as produced by
setup_inputs()) and returns the FULL [1, T] float32 output.  Internally the
work is sharded data-parallel over the leading cluster axis across 8
NeuronCores; the tiny MLP weights are replicated.

Math (per task j, verified against the reference):
    me_j   = mean(enode[j,:])                       # since y41 = y2 * me
    sc_j   = sum(ccl[j,:]) * sum(cnd[j,:])          # since y42 = y2 * sc
    u_j    = [bb_j (768) ; outer3(res_j, fr_j, estep_j) (150)]   # 918 (permuted)
    y2_j   = relu(W1p.T u_j + b1)                   # 128
    a3     = me*(y2@W3)+b3 ; a5 = sc*(y2@W5)+b5     # sigmoid-gated pair
    a4     = me*(y2@W4)+b4 ; a6 = sc*(y2@W6)+b6     # linear pair
    p      = sig(a3)*sig(a5)
    y      = FAILC + p*((a4+a6) - FAILC)

Precision: bb and the bb-part of W1 are bf16 hi+lo pairs; the matmul
accumulates uh@wh + uh@wl + ul@wh (ul@wl negligible).  outer3 features and
their W1 rows are single bf16 (measured ~2.5e-3 rel vs the 2e-2 gate).

Perf notes (from trace analysis):
  - each HWDGE dma_start costs ~600ns of serialized DIRECT2D descriptor
    generation on its sequencer -> merge everything into 9 starts.
  - the PE runs at 1.2GHz (HAM cold) until it has been busy ~3.4us ->
    warm-up matmuls bridge the first DMA wait so the real stream is gapless
    and mostly warm (2.4GHz).
  - DMA floor for the ~2.15MB/core is ~6us; the k-chunk packs are ordered
    so each accumulation matmul's operand arrives just in time.
"""

import sys

if "/opt/trn_rl_repo" not in sys.path:
    sys.path.insert(0, "/opt/trn_rl_repo")

from contextlib import ExitStack

import numpy as np

import concourse.bass as bass
import concourse.mybir as mybir
import concourse.tile as tile
from concourse.bass_utils import run_bass_kernel_spmd

# Problem constants (hardcoded per the harness contract).
NCORES = 8
C, TASKS = 64, 64
T = C * TASKS                 # 4096
TC = T // NCORES              # 512 tasks per core
D_BB = 768
N_OUT = 150                   # 5*5*6 outer-product features
D_H = 128
E_N = 64
C_C, C_N = 4, 32
FAILC = -100.0
NTILE = TC // 128             # 4 task tiles of 128 per core
NBB = D_BB // 128             # 6 bb k-chunks
N_WARM = 6                    # PE warm-up matmuls while first DMAs land

F32 = mybir.dt.float32
BF16 = mybir.dt.bfloat16


def _build_module():
    nc = bass.Bass()

    # w1hi cols: [0:768) bb-hi w1 chunks, [768:896) o3a w1 rows
    w1hi = nc.declare_dram_parameter("w1hi", [128, 896], BF16, isOutput=False)
    w1lo = nc.declare_dram_parameter("w1lo", [128, 768], BF16, isOutput=False)
    # u k-chunk packs, in arrival order: o3a, uh0..uh5, ul0..ul5
    upkA = nc.declare_dram_parameter("upkA", [128, 3, TC], BF16, isOutput=False)
    upkB = nc.declare_dram_parameter("upkB", [128, 3, TC], BF16, isOutput=False)
    upkC = nc.declare_dram_parameter("upkC", [128, 3, TC], BF16, isOutput=False)
    upkD = nc.declare_dram_parameter("upkD", [128, 4, TC], BF16, isOutput=False)
    # o3t cols: [0:128) W1 rows for o3b, [128:640) o3b features, [640:768) b1
    o3t = nc.declare_dram_parameter("o3t", [22, 768], BF16, isOutput=False)
    mesc4 = nc.declare_dram_parameter("mesc4", [128, NTILE, 4], F32, isOutput=False)
    # misc9 cols: [0:4) wh (W3,W5,W4,W6), [4:8) bh' (b3,b5,b4,b6-FAILC), [8:9) b1
    misc9 = nc.declare_dram_parameter("misc9", [128, 9], F32, isOutput=False)
    out = nc.declare_dram_parameter("out", [128, NTILE], F32, isOutput=True)

    with tile.TileContext(nc) as tc, ExitStack() as ctx:
        pool = ctx.enter_context(tc.tile_pool(name="main", bufs=1))
        psum = ctx.enter_context(tc.tile_pool(name="psum", bufs=1, space="PSUM"))

        # PE warm-up tile (HAM): dummy matmuls keep the PE clock at 2.4GHz.
        wz = pool.tile([128, TC], BF16, tag="wz")
        nc.vector.memset(wz, 0.0)
        pwz = psum.tile([128, TC], F32, tag="pwz")

        def warm(n):
            for _ in range(n):
                nc.tensor.matmul(pwz, lhsT=wz[:, 0:D_H], rhs=wz, start=True, stop=True)

        # ---- big loads on the sync HWDGE ring, in consumption order -------
        w1h_s = pool.tile([128, 896], BF16, tag="w1h")
        nc.sync.dma_start(out=w1h_s, in_=w1hi[:, :])
        ups = []
        for name, par, nslot in (("A", upkA, 3), ("B", upkB, 3),
                                 ("C", upkC, 3), ("D", upkD, 4)):
            t = pool.tile([128, nslot, TC], BF16, tag=f"up{name}")
            nc.sync.dma_start(out=t, in_=par[:, :, :])
            ups.append(t)
        upA, upB, upC, upD = ups

        # ---- small loads: scalar HWDGE ring + gpsimd SWDGE ----------------
        o3t_s = pool.tile([22, 768], BF16, tag="o3t")
        nc.scalar.dma_start(out=o3t_s, in_=o3t[:, :])
        w1l_s = pool.tile([128, 768], BF16, tag="w1l")
        nc.scalar.dma_start(out=w1l_s, in_=w1lo[:, :])
        misc_s = pool.tile([128, 9], F32, tag="misc")
        nc.scalar.dma_start(out=misc_s, in_=misc9[:, :])
        mesc_s = pool.tile([128, NTILE, 4], F32, tag="mesc")
        nc.gpsimd.dma_start(out=mesc_s, in_=mesc4[:, :, :])

        # Preload the sigmoid ACT table (after the scalar DMAs so they are
        # not blocked behind the ~1.3us table load).
        sgw = pool.tile([32, 1], F32, tag="sgw")
        nc.vector.memset(sgw, 0.0)
        ones1 = pool.tile([1, TC], BF16, tag="ones1")
        nc.vector.memset(ones1, 1.0)
        nc.scalar.activation(sgw, sgw, mybir.ActivationFunctionType.Sigmoid)

        # ---- main matmul: psumY = W1h.T uh + W1l.T uh + W1h.T ul ----------
        psumY = psum.tile([128, TC], F32, tag="psumY")
        n_mm = 3 * NBB + 3
        pos = 0

        def mm(lhsT, rhs):
            nonlocal pos
            nc.tensor.matmul(
                psumY, lhsT=lhsT, rhs=rhs,
                start=(pos == 0), stop=(pos == n_mm - 1))
            pos += 1

        def w1h(j):        # bb-hi chunk j (0..5)
            return w1h_s[:, 128 * j : 128 * (j + 1)]

        def w1l(j):        # bb-lo chunk j (0..5)
            return w1l_s[:, 128 * j : 128 * (j + 1)]

        # chunk layout: upA = (o3a, uh0, uh1), upB = (uh2, uh3, uh4),
        # upC = (uh5, ul0, ul1), upD = (ul2..ul5)
        uh_sl = [upA[:, 1, :], upA[:, 2, :], upB[:, 0, :],
                 upB[:, 1, :], upB[:, 2, :], upC[:, 0, :]]
        ul_sl = [upC[:, 1, :], upC[:, 2, :], upD[:, 0, :],
                 upD[:, 1, :], upD[:, 2, :], upD[:, 3, :]]

        warm(5)
        mm(w1h_s[:, 768:896], upA[:, 0, :])     # o3a (upA)
        mm(w1h(0), uh_sl[0])
        mm(w1h(1), uh_sl[1])
        mm(o3t_s[:, 0:128], o3t_s[:, 128:640])  # o3b (k=22), arrives ~with upB
        mm(o3t_s[0:1, 640:768], ones1)          # b1 via k=1 ones-row matmul
        mm(w1h(2), uh_sl[2])
        mm(w1h(3), uh_sl[3])
        mm(w1h(4), uh_sl[4])
        mm(w1l(0), uh_sl[0])                    # wl: no new DMA needed
        mm(w1l(1), uh_sl[1])
        mm(w1h(5), uh_sl[5])
        mm(w1l(2), uh_sl[2])
        mm(w1l(3), uh_sl[3])
        mm(w1l(4), uh_sl[4])
        mm(w1l(5), uh_sl[5])
        for j in range(NBB):
            mm(w1h(j), ul_sl[j])                # ul@wh: paced by the ul DMAs

        # ---- relu in halves on two engines (ACT + DVE run in parallel);
        # b1 is already accumulated in psum via the ones-row matmul.
        y2T = pool.tile([128, TC], F32, tag="y2T")
        nc.scalar.activation(
            y2T[:, 0:256], psumY[:, 0:256],
            mybir.ActivationFunctionType.Relu)
        nc.vector.tensor_scalar_max(y2T[:, 256:512], psumY[:, 256:512], 0.0)

        # ---- heads, task-major: one 128-task tile at a time --------------
        # cols of psumS[:, i, :]: d3, d5, d4, d6  (W3, W5, W4, W6 order)
        psumS = psum.tile([128, NTILE, 4], F32, tag="psumS")
        for i in range(NTILE):
            nc.tensor.matmul(
                psumS[:, i, :],
                lhsT=y2T[:, 128 * i : 128 * (i + 1)],
                rhs=misc_s[:, 0:4],
                start=True,
                stop=True,
            )

        # ---- combine ------------------------------------------------------
        # bh' folds -FAILC into the b6 column, so y6s = a4 + a6 - FAILC and
        # out = pv*y6s + FAILC.
        av = pool.tile([128, NTILE, 4], F32, tag="av")
        nc.vector.tensor_mul(av, psumS, mesc_s)
        nc.vector.tensor_add(
            av, av, misc_s[:, 4:8].unsqueeze(1).broadcast_to([128, NTILE, 4])
        )
        y6s = pool.tile([128, NTILE, 1], F32, tag="y6s")
        nc.vector.tensor_add(y6s, av[:, :, 2:3], av[:, :, 3:4])

        sg = pool.tile([128, NTILE, 2], F32, tag="sg")
        nc.scalar.activation(sg, av[:, :, 0:2], mybir.ActivationFunctionType.Sigmoid)

        pv = pool.tile([128, NTILE, 1], F32, tag="pv")
        nc.vector.tensor_mul(pv, sg[:, :, 0:1], sg[:, :, 1:2])
        om = pool.tile([128, NTILE, 1], F32, tag="om")
        nc.vector.tensor_mul(om, pv, y6s)
        outv = pool.tile([128, NTILE, 1], F32, tag="outv")
        nc.vector.tensor_scalar_add(outv, om, FAILC)

        nc.sync.dma_start(out=out[:, :], in_=outv[:, :, 0])

    return _split_sync_waits(nc)


def _split_sync_waits(nc, max_waits=1):
    """This container's walrus rejects >1 sem-wait per instruction
    ("Too many sync wait commands"); hoist extras onto same-engine NOPs."""
    nid = 0
    for f in nc.m.functions:
        for bb in f.blocks:
            new = []
            for inst in bb.instructions:
                si = inst.sync_info
                if si is None:
                    new.append(inst)
                    continue
                waits = list(si.on_wait or [])
                if len(waits) > max_waits:
                    for w in waits[:-max_waits]:
                        nop = mybir.InstNoOp(name=f"WSPL-{nid}", ins=[], outs=[])
                        nid += 1
                        nop.engine = inst.engine
                        nop.sync_info = mybir.SyncInfo(on_wait=[w], on_update=[])
                        new.append(nop)
                    inst.sync_info = mybir.SyncInfo(
                        on_wait=waits[-max_waits:], on_update=list(si.on_update or [])
                    )
                new.append(inst)
            bb.instructions = new
    return nc


_CACHED_NC = None


def _get_nc():
    global _CACHED_NC
    if _CACHED_NC is None:
        _CACHED_NC = _build_module()
    return _CACHED_NC


def _make_in_maps(inputs: dict) -> list[dict[str, np.ndarray]]:
    f32 = np.float32
    bf16 = np.dtype("bfloat16")

    bb = np.asarray(inputs["backbone_y"], f32).reshape(T, D_BB)
    res = np.asarray(inputs["y_res"], f32).reshape(T, 5)
    fr = np.asarray(inputs["y_fr"], f32).reshape(T, 5)
    estep = np.asarray(inputs["y_estep"], f32).reshape(T, 6)
    enode = np.asarray(inputs["y_enode"], f32).reshape(T, E_N)
    ccl = np.asarray(inputs["y_ccluster"], f32).reshape(T, C_C)
    cnd = np.asarray(inputs["y_cnode"], f32).reshape(T, C_N)

    # outer3 features [T, 150] and per-task scalars (host precompute)
    o3 = np.einsum("tn,tm,to->tnmo", res, fr, estep).reshape(T, N_OUT)
    me = enode.mean(axis=1).astype(f32)
    sc = (ccl.sum(axis=1) * cnd.sum(axis=1)).astype(f32)

    # W1 packed with permuted rows: [bb (768) ; outer3 (150)]
    w1 = np.ascontiguousarray(np.asarray(inputs["W1"], f32))
    w1a = w1[0:N_OUT]        # outer3 rows
    w1b = w1[N_OUT:]         # bb rows [768, 128]
    w1bh = w1b.astype(bf16)
    w1bl = (w1b - w1bh.astype(f32)).astype(bf16)
    w1hi_c = np.ascontiguousarray(
        np.concatenate(
            [
                w1bh.reshape(NBB, 128, D_H).transpose(1, 0, 2).reshape(128, NBB * D_H),
                w1a[0:128].astype(bf16),
            ],
            axis=1,
        )
    )  # [128, 896]
    w1lo_c = np.ascontiguousarray(
        w1bl.reshape(NBB, 128, D_H).transpose(1, 0, 2).reshape(128, NBB * D_H)
    )  # [128, 768]
    b1_col = np.asarray(inputs["b1"], f32).reshape(D_H, 1)

    w3 = np.asarray(inputs["W3"], f32).reshape(D_H, 1)
    w4 = np.asarray(inputs["W4"], f32).reshape(D_H, 1)
    w5 = np.asarray(inputs["W5"], f32).reshape(D_H, 1)
    w6 = np.asarray(inputs["W6"], f32).reshape(D_H, 1)
    bh_row = np.array(
        [
            float(np.asarray(inputs["b3"]).reshape(-1)[0]),
            float(np.asarray(inputs["b5"]).reshape(-1)[0]),
            float(np.asarray(inputs["b4"]).reshape(-1)[0]),
            float(np.asarray(inputs["b6"]).reshape(-1)[0]) - FAILC,
        ],
        f32,
    )
    # [128, 9]: wh | bh' | b1
    misc9_c = np.ascontiguousarray(
        np.concatenate(
            [
                np.concatenate([w3, w5, w4, w6], axis=1),
                np.broadcast_to(bh_row, (128, 4)),
                b1_col,
            ],
            axis=1,
        )
    )

    in_maps = []
    for c in range(NCORES):
        sl = slice(c * TC, (c + 1) * TC)
        bbT = bb[sl].T                       # [768, TC]
        uh_c = bbT.astype(bf16)              # C-contiguous, [6*128, TC]
        ul_c = (bbT - uh_c.astype(f32)).astype(bf16)
        o3T = o3[sl].T.astype(bf16)          # [150, TC]
        # packs in arrival order: o3a, uh0..5, ul0..5
        chunks = [o3T[0:128]] + [uh_c[128 * j : 128 * (j + 1)] for j in range(NBB)] \
            + [ul_c[128 * j : 128 * (j + 1)] for j in range(NBB)]
        st = np.stack(chunks, axis=1)        # [128, 13, TC]
        o3t_c = np.ascontiguousarray(
            np.concatenate(
                [
                    w1a[128:N_OUT].astype(bf16),
                    o3T[128:N_OUT],
                    np.broadcast_to(b1_col.reshape(1, D_H), (22, D_H)).astype(bf16),
                ],
                axis=1,
            )
        )  # [22, 768]
        mesc_c = np.ascontiguousarray(
            np.stack(
                [me[sl].reshape(NTILE, 128).T, sc[sl].reshape(NTILE, 128).T] * 2,
                axis=-1,
            )
        )  # [128, NTILE, 4] = me, sc, me, sc
        in_maps.append(
            {
                "w1hi": w1hi_c,
                "w1lo": w1lo_c,
                "upkA": np.ascontiguousarray(st[:, 0:3]),
                "upkB": np.ascontiguousarray(st[:, 3:6]),
                "upkC": np.ascontiguousarray(st[:, 6:9]),
                "upkD": np.ascontiguousarray(st[:, 9:13]),
                "o3t": o3t_c,
                "mesc4": mesc_c,
                "misc9": misc9_c,
            }
        )
    return in_maps


def _assemble(results: list[dict[str, np.ndarray]]) -> np.ndarray:
    parts = [np.asarray(results[c]["out"]).T.reshape(-1) for c in range(NCORES)]
    return np.concatenate(parts)[None, :].astype(np.float32)


def _run(inputs: dict, trace: bool = False):
    nc = _get_nc()
    in_maps = _make_in_maps(inputs)
    kres = run_bass_kernel_spmd(
        nc, in_maps, core_ids=list(range(NCORES)), trace=trace
    )
    return _assemble(kres.results), kres


def kernel(**inputs) -> np.ndarray:
    out, _ = _run(inputs)
    return out


# revision 10
# speedup vs baseline: 1.2247x; 1.1962x over previous
"""Trainium2 Bass kernel for nn_CriticHead (critic head over C*t tasks).

Contract: kernel(**inputs) takes the FULL unsharded inputs (as produced by
setup_inputs()) and returns the FULL [1, T] float32 output.  Internally the
work is sharded data-parallel over the leading cluster axis across 8
NeuronCores; the tiny MLP weights are replicated.

Math (per task j, verified against the reference):
    me_j   = mean(enode[j,:])                       # since y41 = y2 * me
    sc_j   = sum(ccl[j,:]) * sum(cnd[j,:])          # since y42 = y2 * sc
    u_j    = [bb_j (768) ; outer3(res_j, fr_j, estep_j) (150)]   # 918 (permuted)
    y2_j   = relu(W1p.T u_j + b1)                   # 128
    a3     = me*(y2@W3)+b3 ; a5 = sc*(y2@W5)+b5     # sigmoid-gated pair
    a4     = me*(y2@W4)+b4 ; a6 = sc*(y2@W6)+b6     # linear pair
    p      = sig(a3)*sig(a5)
    y      = FAILC + p*((a4+a6) - FAILC)

Precision: u and W1 are single fp16 (10-bit mantissa suits the N(0,1)
backbone data; measured 2.7e-3 rel vs the 2e-2 gate).  The f32 head path
(y2T, wh) is required -- bf16 anywhere in the head path fails the gate.
b1 is folded into the PSUM accumulation via a k=1 ones-row matmul so both
relu halves are a pure max(x, 0).

Perf notes (from trace analysis):
  - each HWDGE dma_start costs ~0.6-0.7us of serialized DIRECT2D descriptor
    generation on its sequencer -> few, large, contiguous-per-partition
    transfers, split across the sync/scalar/gpsimd generators.
  - the PE runs at 1.2GHz (HAM cold) until it has been busy ~3.4us; the
    front warm-up matmuls bridge the first DMA wait.
  - each engine executes its queue in order: matmuls are emitted in
    expected data-arrival order (o3t -> upA -> upB -> upC).
"""

import sys

if "/opt/trn_rl_repo" not in sys.path:
    sys.path.insert(0, "/opt/trn_rl_repo")

from contextlib import ExitStack

import numpy as np

import concourse.bass as bass
import concourse.mybir as mybir
import concourse.tile as tile
from concourse.bass_utils import run_bass_kernel_spmd

# Problem constants (hardcoded per the harness contract).
NCORES = 8
C, TASKS = 64, 64
T = C * TASKS                 # 4096
TC = T // NCORES              # 512 tasks per core
D_BB = 768
N_OUT = 150                   # 5*5*6 outer-product features
D_H = 128
E_N = 64
C_C, C_N = 4, 32
FAILC = -100.0
NTILE = TC // 128             # 4 task tiles of 128 per core
NBB = D_BB // 128             # 6 bb k-chunks
N_WARM = 5                    # PE warm-up matmuls while first DMAs land

F32 = mybir.dt.float32
F16 = mybir.dt.float16


def _build_module():
    nc = bass.Bass()

    # w1f cols: [0:768) bb w1 chunks, [768:896) o3a w1 rows (fp16)
    w1f = nc.declare_dram_parameter("w1f", [128, 896], F16, isOutput=False)
    # u k-chunk packs, in arrival order: o3a, uh0..uh5 (fp16)
    upkA = nc.declare_dram_parameter("upkA", [128, 2, TC], F16, isOutput=False)
    upkB = nc.declare_dram_parameter("upkB", [128, 2, TC], F16, isOutput=False)
    upkC = nc.declare_dram_parameter("upkC", [128, 3, TC], F16, isOutput=False)
    # o3t cols: [0:128) W1 rows for o3b, [128:640) o3b features, [640:768) b1
    o3t = nc.declare_dram_parameter("o3t", [22, 768], F16, isOutput=False)
    mesc4 = nc.declare_dram_parameter("mesc4", [128, NTILE, 4], F32, isOutput=False)
    # misc9 cols: [0:4) wh (W3,W5,W4,W6), [4:8) bh' (b3,b5,b4,b6-FAILC), [8:9) b1
    misc9 = nc.declare_dram_parameter("misc9", [128, 9], F32, isOutput=False)
    out = nc.declare_dram_parameter("out", [128, NTILE], F32, isOutput=True)

    with tile.TileContext(nc) as tc, ExitStack() as ctx:
        pool = ctx.enter_context(tc.tile_pool(name="main", bufs=1))
        psum = ctx.enter_context(tc.tile_pool(name="psum", bufs=1, space="PSUM"))

        # PE warm-up tile (HAM): dummy matmuls keep the PE clock at 2.4GHz.
        wz = pool.tile([128, TC], F16, tag="wz")
        nc.vector.memset(wz, 0.0)
        pwz = psum.tile([128, TC], F32, tag="pwz")

        def warm(n):
            for _ in range(n):
                nc.tensor.matmul(pwz, lhsT=wz[:, 0:D_H], rhs=wz, start=True, stop=True)

        # ---- big loads on the sync HWDGE ring, in consumption order -------
        w1_s = pool.tile([128, 896], F16, tag="w1")
        nc.sync.dma_start(out=w1_s, in_=w1f[:, :])
        ups = []
        for name, par, nslot in (("A", upkA, 2), ("B", upkB, 2), ("C", upkC, 3)):
            t = pool.tile([128, nslot, TC], F16, tag=f"up{name}")
            nc.sync.dma_start(out=t, in_=par[:, :, :])
            ups.append(t)
        upA, upB, upC = ups

        # ---- small loads: scalar HWDGE ring + gpsimd SWDGE ----------------
        o3t_s = pool.tile([22, 768], F16, tag="o3t")
        nc.scalar.dma_start(out=o3t_s, in_=o3t[:, :])
        misc_s = pool.tile([128, 9], F32, tag="misc")
        nc.scalar.dma_start(out=misc_s, in_=misc9[:, :])
        mesc_s = pool.tile([128, NTILE, 4], F32, tag="mesc")
        nc.gpsimd.dma_start(out=mesc_s, in_=mesc4[:, :, :])

        # Preload the sigmoid ACT table (after the scalar DMAs so they are
        # not blocked behind the ~1.3us table load).
        sgw = pool.tile([32, 1], F32, tag="sgw")
        nc.vector.memset(sgw, 0.0)
        ones1 = pool.tile([1, TC], F16, tag="ones1")
        nc.vector.memset(ones1, 1.0)
        nc.scalar.activation(sgw, sgw, mybir.ActivationFunctionType.Sigmoid)

        # ---- main matmul: psumY = W1.T u + b1 -----------------------------
        psumY = psum.tile([128, TC], F32, tag="psumY")
        n_mm = NBB + 3
        pos = 0

        def mm(lhsT, rhs):
            nonlocal pos
            nc.tensor.matmul(
                psumY, lhsT=lhsT, rhs=rhs,
                start=(pos == 0), stop=(pos == n_mm - 1))
            pos += 1

        # chunk layout: upA = (o3a, uh0), upB = (uh1, uh2), upC = (uh3..uh5)
        uh_sl = [upA[:, 1, :], upB[:, 0, :], upB[:, 1, :],
                 upC[:, 0, :], upC[:, 1, :], upC[:, 2, :]]

        warm(N_WARM)
        mm(o3t_s[:, 0:128], o3t_s[:, 128:640])  # o3b (k=22), scalar ring
        mm(o3t_s[0:1, 640:768], ones1)          # b1 via k=1 ones-row matmul
        mm(w1_s[:, 768:896], upA[:, 0, :])      # o3a
        for j in range(NBB):
            mm(w1_s[:, 128 * j : 128 * (j + 1)], uh_sl[j])

        # ---- relu in halves on two engines (ACT + DVE run in parallel);
        # b1 is already accumulated in psum via the ones-row matmul.
        y2T = pool.tile([128, TC], F32, tag="y2T")
        nc.scalar.activation(
            y2T[:, 0:256], psumY[:, 0:256],
            mybir.ActivationFunctionType.Relu)
        nc.vector.tensor_scalar_max(y2T[:, 256:512], psumY[:, 256:512], 0.0)

        # ---- heads, task-major: one 128-task tile at a time --------------
        # cols of psumS[:, i, :]: d3, d5, d4, d6  (W3, W5, W4, W6 order)
        psumS = psum.tile([128, NTILE, 4], F32, tag="psumS")
        for i in range(NTILE):
            nc.tensor.matmul(
                psumS[:, i, :],
                lhsT=y2T[:, 128 * i : 128 * (i + 1)],
                rhs=misc_s[:, 0:4],
                start=True,
                stop=True,
            )

        # ---- combine ------------------------------------------------------
        # bh' folds -FAILC into the b6 column, so y6s = a4 + a6 - FAILC and
        # out = pv*y6s + FAILC.
        av = pool.tile([128, NTILE, 4], F32, tag="av")
        nc.vector.tensor_mul(av, psumS, mesc_s)
        nc.vector.tensor_add(
            av, av, misc_s[:, 4:8].unsqueeze(1).broadcast_to([128, NTILE, 4])
        )
        y6s = pool.tile([128, NTILE, 1], F32, tag="y6s")
        nc.vector.tensor_add(y6s, av[:, :, 2:3], av[:, :, 3:4])

        sg = pool.tile([128, NTILE, 2], F32, tag="sg")
        nc.scalar.activation(sg, av[:, :, 0:2], mybir.ActivationFunctionType.Sigmoid)

        pv = pool.tile([128, NTILE, 1], F32, tag="pv")
        nc.vector.tensor_mul(pv, sg[:, :, 0:1], sg[:, :, 1:2])
        om = pool.tile([128, NTILE, 1], F32, tag="om")
        nc.vector.tensor_mul(om, pv, y6s)
        outv = pool.tile([128, NTILE, 1], F32, tag="outv")
        nc.vector.tensor_scalar_add(outv, om, FAILC)

        nc.sync.dma_start(out=out[:, :], in_=outv[:, :, 0])

    return _split_sync_waits(nc)


def _split_sync_waits(nc, max_waits=1):
    """This container's walrus rejects >1 sem-wait per instruction
    ("Too many sync wait commands"); hoist extras onto same-engine NOPs."""
    nid = 0
    for f in nc.m.functions:
        for bb in f.blocks:
            new = []
            for inst in bb.instructions:
                si = inst.sync_info
                if si is None:
                    new.append(inst)
                    continue
                waits = list(si.on_wait or [])
                if len(waits) > max_waits:
                    for w in waits[:-max_waits]:
                        nop = mybir.InstNoOp(name=f"WSPL-{nid}", ins=[], outs=[])
                        nid += 1
                        nop.engine = inst.engine
                        nop.sync_info = mybir.SyncInfo(on_wait=[w], on_update=[])
                        new.append(nop)
                    inst.sync_info = mybir.SyncInfo(
                        on_wait=waits[-max_waits:], on_update=list(si.on_update or [])
                    )
                new.append(inst)
            bb.instructions = new
    return nc


_CACHED_NC = None


def _get_nc():
    global _CACHED_NC
    if _CACHED_NC is None:
        _CACHED_NC = _build_module()
    return _CACHED_NC


def _make_in_maps(inputs: dict) -> list[dict[str, np.ndarray]]:
    f32 = np.float32
    f16 = np.float16

    bb = np.asarray(inputs["backbone_y"], f32).reshape(T, D_BB)
    res = np.asarray(inputs["y_res"], f32).reshape(T, 5)
    fr = np.asarray(inputs["y_fr"], f32).reshape(T, 5)
    estep = np.asarray(inputs["y_estep"], f32).reshape(T, 6)
    enode = np.asarray(inputs["y_enode"], f32).reshape(T, E_N)
    ccl = np.asarray(inputs["y_ccluster"], f32).reshape(T, C_C)
    cnd = np.asarray(inputs["y_cnode"], f32).reshape(T, C_N)

    # outer3 features [T, 150] and per-task scalars (host precompute)
    o3 = np.einsum("tn,tm,to->tnmo", res, fr, estep).reshape(T, N_OUT)
    me = enode.mean(axis=1).astype(f32)
    sc = (ccl.sum(axis=1) * cnd.sum(axis=1)).astype(f32)

    # W1 packed fp16 with permuted rows: [bb (768) ; outer3 (150)]
    w1 = np.ascontiguousarray(np.asarray(inputs["W1"], f32))
    w1a = w1[0:N_OUT]        # outer3 rows
    w1b = w1[N_OUT:]         # bb rows [768, 128]
    w1f_c = np.ascontiguousarray(
        np.concatenate(
            [
                w1b.astype(f16).reshape(NBB, 128, D_H).transpose(1, 0, 2)
                .reshape(128, NBB * D_H),
                w1a[0:128].astype(f16),
            ],
            axis=1,
        )
    )  # [128, 896]
    b1_col = np.asarray(inputs["b1"], f32).reshape(D_H, 1)

    w3 = np.asarray(inputs["W3"], f32).reshape(D_H, 1)
    w4 = np.asarray(inputs["W4"], f32).reshape(D_H, 1)
    w5 = np.asarray(inputs["W5"], f32).reshape(D_H, 1)
    w6 = np.asarray(inputs["W6"], f32).reshape(D_H, 1)
    bh_row = np.array(
        [
            float(np.asarray(inputs["b3"]).reshape(-1)[0]),
            float(np.asarray(inputs["b5"]).reshape(-1)[0]),
            float(np.asarray(inputs["b4"]).reshape(-1)[0]),
            float(np.asarray(inputs["b6"]).reshape(-1)[0]) - FAILC,
        ],
        f32,
    )
    # [128, 9]: wh | bh' | b1
    misc9_c = np.ascontiguousarray(
        np.concatenate(
            [
                np.concatenate([w3, w5, w4, w6], axis=1),
                np.broadcast_to(bh_row, (128, 4)),
                b1_col,
            ],
            axis=1,
        )
    )

    in_maps = []
    for c in range(NCORES):
        sl = slice(c * TC, (c + 1) * TC)
        uh_c = bb[sl].T.astype(f16)          # [768, TC], C-contiguous
        o3T = o3[sl].T.astype(f16)           # [150, TC]
        # packs in arrival order: o3a, uh0..5
        chunks = [o3T[0:128]] + [uh_c[128 * j : 128 * (j + 1)] for j in range(NBB)]
        st = np.stack(chunks, axis=1)        # [128, 7, TC]
        o3t_c = np.ascontiguousarray(
            np.concatenate(
                [
                    w1a[128:N_OUT].astype(f16),
                    o3T[128:N_OUT],
                    np.broadcast_to(b1_col.reshape(1, D_H), (22, D_H)).astype(f16),
                ],
                axis=1,
            )
        )  # [22, 768]
        mesc_c = np.ascontiguousarray(
            np.stack(
                [me[sl].reshape(NTILE, 128).T, sc[sl].reshape(NTILE, 128).T] * 2,
                axis=-1,
            )
        )  # [128, NTILE, 4] = me, sc, me, sc
        in_maps.append(
            {
                "w1f": w1f_c,
                "upkA": np.ascontiguousarray(st[:, 0:2]),
                "upkB": np.ascontiguousarray(st[:, 2:4]),
                "upkC": np.ascontiguousarray(st[:, 4:7]),
                "o3t": o3t_c,
                "mesc4": mesc_c,
                "misc9": misc9_c,
            }
        )
    return in_maps


def _assemble(results: list[dict[str, np.ndarray]]) -> np.ndarray:
    parts = [np.asarray(results[c]["out"]).T.reshape(-1) for c in range(NCORES)]
    return np.concatenate(parts)[None, :].astype(np.float32)


def _run(inputs: dict, trace: bool = False):
    nc = _get_nc()
    in_maps = _make_in_maps(inputs)
    kres = run_bass_kernel_spmd(
        nc, in_maps, core_ids=list(range(NCORES)), trace=trace
    )
    return _assemble(kres.results), kres


def kernel(**inputs) -> np.ndarray:
    out, _ = _run(inputs)
    return out


# revision 11
# speedup vs baseline: 1.2940x; 1.0565x over previous
"""Trainium2 Bass kernel for nn_CriticHead (critic head over C*t tasks).

Contract: kernel(**inputs) takes the FULL unsharded inputs (as produced by
setup_inputs()) and returns the FULL [1, T] float32 output.  Internally the
work is sharded data-parallel over the leading cluster axis across 8
NeuronCores; the tiny MLP weights are replicated.

Math (per task j, verified against the reference):
    me_j   = mean(enode[j,:])                       # since y41 = y2 * me
    sc_j   = sum(ccl[j,:]) * sum(cnd[j,:])          # since y42 = y2 * sc
    u_j    = [bb_j (768) ; outer3(res_j, fr_j, estep_j) (150)]   # 918 (permuted)
    y2_j   = relu(W1p.T u_j + b1)                   # 128
    a3     = me*(y2@W3)+b3 ; a5 = sc*(y2@W5)+b5     # sigmoid-gated pair
    a4     = me*(y2@W4)+b4 ; a6 = sc*(y2@W6)+b6     # linear pair
    p      = sig(a3)*sig(a5)
    y      = FAILC + p*((a4+a6) - FAILC)

Precision: u and W1 are single fp16 (10-bit mantissa suits the N(0,1)
backbone data; measured 2.7e-3 rel vs the 2e-2 gate).  The f32 head path
(y2T, wh) is required -- bf16 anywhere in the head path fails the gate.
b1 is folded into the PSUM accumulation via a k=1 ones-row matmul so both
relu halves are a pure max(x, 0).

Perf notes (from trace analysis):
  - each HWDGE dma_start costs ~0.6-0.7us of serialized DIRECT2D descriptor
    generation on its sequencer -> few, large, contiguous-per-partition
    transfers, split across the sync/scalar/gpsimd generators.
  - the PE runs at 1.2GHz (HAM cold) until it has been busy ~3.4us; the
    front warm-up matmuls bridge the first DMA wait.
  - each engine executes its queue in order: matmuls are emitted in
    expected data-arrival order (o3t -> upA -> upB -> upC).
"""

import sys

if "/opt/trn_rl_repo" not in sys.path:
    sys.path.insert(0, "/opt/trn_rl_repo")

from contextlib import ExitStack

import numpy as np

import concourse.bass as bass
import concourse.mybir as mybir
import concourse.tile as tile
from concourse.bass_utils import run_bass_kernel_spmd

# Problem constants (hardcoded per the harness contract).
NCORES = 8
C, TASKS = 64, 64
T = C * TASKS                 # 4096
TC = T // NCORES              # 512 tasks per core
D_BB = 768
N_OUT = 150                   # 5*5*6 outer-product features
D_H = 128
E_N = 64
C_C, C_N = 4, 32
FAILC = -100.0
NTILE = TC // 128             # 4 task tiles of 128 per core
NBB = D_BB // 128             # 6 bb k-chunks
N_WARM = 8                    # PE warm-up matmuls while first DMAs land

F32 = mybir.dt.float32
F16 = mybir.dt.float16


def _build_module():
    nc = bass.Bass()

    # w1f cols: [0:768) bb w1 chunks, [768:896) o3a w1 rows (fp16)
    w1f = nc.declare_dram_parameter("w1f", [128, 896], F16, isOutput=False)
    # u k-chunk packs, in arrival order: o3a, uh0..uh5 (fp16)
    # upkA cols: [0:TC) o3a, [TC:2TC) uh0, [2TC:2TC+4) wh fp16 (W3,W5,W4,W6)
    upkA = nc.declare_dram_parameter("upkA", [128, 2 * TC + 4], F16, isOutput=False)
    upkB = nc.declare_dram_parameter("upkB", [128, 2, TC], F16, isOutput=False)
    upkC = nc.declare_dram_parameter("upkC", [128, 3, TC], F16, isOutput=False)
    # o3t cols: [0:128) W1 rows for o3b, [128:640) o3b features, [640:768) b1
    o3t = nc.declare_dram_parameter("o3t", [22, 768], F16, isOutput=False)
    mesc4 = nc.declare_dram_parameter("mesc4", [128, NTILE, 4], F32, isOutput=False)
    # misc9 cols: [0:4) wh (W3,W5,W4,W6), [4:8) bh' (b3,b5,b4,b6-FAILC), [8:9) b1
    misc9 = nc.declare_dram_parameter("misc9", [128, 9], F32, isOutput=False)
    out = nc.declare_dram_parameter("out", [128, NTILE], F32, isOutput=True)

    with tile.TileContext(nc) as tc, ExitStack() as ctx:
        pool = ctx.enter_context(tc.tile_pool(name="main", bufs=1))
        psum = ctx.enter_context(tc.tile_pool(name="psum", bufs=1, space="PSUM"))

        # PE warm-up tile (HAM): dummy matmuls keep the PE clock at 2.4GHz.
        wz = pool.tile([128, TC], F16, tag="wz")
        nc.vector.memset(wz, 0.0)
        pwz = psum.tile([128, TC], F32, tag="pwz")

        def warm(n):
            for _ in range(n):
                nc.tensor.matmul(pwz, lhsT=wz[:, 0:D_H], rhs=wz, start=True, stop=True)

        # ---- big loads on the sync HWDGE ring, in consumption order -------
        w1_s = pool.tile([128, 896], F16, tag="w1")
        nc.sync.dma_start(out=w1_s, in_=w1f[:, :])
        upA = pool.tile([128, 2 * TC + 4], F16, tag="upA")
        nc.sync.dma_start(out=upA, in_=upkA[:, :])
        ups = []
        for name, par, nslot in (("B", upkB, 2), ("C", upkC, 3)):
            t = pool.tile([128, nslot, TC], F16, tag=f"up{name}")
            nc.sync.dma_start(out=t, in_=par[:, :, :])
            ups.append(t)
        upB, upC = ups

        # ---- small loads: scalar HWDGE ring + gpsimd SWDGE ----------------
        o3t_s = pool.tile([22, 768], F16, tag="o3t")
        nc.scalar.dma_start(out=o3t_s, in_=o3t[:, :])
        misc_s = pool.tile([128, 9], F32, tag="misc")
        nc.scalar.dma_start(out=misc_s, in_=misc9[:, :])
        mesc_s = pool.tile([128, NTILE, 4], F32, tag="mesc")
        nc.gpsimd.dma_start(out=mesc_s, in_=mesc4[:, :, :])

        # Preload the sigmoid ACT table (after the scalar DMAs so they are
        # not blocked behind the ~1.3us table load).
        sgw = pool.tile([32, 1], F32, tag="sgw")
        nc.vector.memset(sgw, 0.0)
        ones1 = pool.tile([1, TC], F16, tag="ones1")
        nc.vector.memset(ones1, 1.0)
        nc.scalar.activation(sgw, sgw, mybir.ActivationFunctionType.Sigmoid)

        # ---- main matmul: psumY = W1.T u + b1 -----------------------------
        psumY = psum.tile([128, TC], F32, tag="psumY")
        n_mm = NBB + 3
        pos = 0

        def mm(lhsT, rhs):
            nonlocal pos
            nc.tensor.matmul(
                psumY, lhsT=lhsT, rhs=rhs,
                start=(pos == 0), stop=(pos == n_mm - 1))
            pos += 1

        # chunk layout: upA = (o3a, uh0, wh), upB = (uh1, uh2), upC = (uh3..uh5)
        uh_sl = [upA[:, TC : 2 * TC], upB[:, 0, :], upB[:, 1, :],
                 upC[:, 0, :], upC[:, 1, :], upC[:, 2, :]]

        warm(N_WARM)
        mm(o3t_s[:, 0:128], o3t_s[:, 128:640])  # o3b (k=22), scalar ring
        mm(o3t_s[0:1, 640:768], ones1)          # b1 via k=1 ones-row matmul
        mm(w1_s[:, 768:896], upA[:, 0:TC])      # o3a
        for j in range(NBB):
            mm(w1_s[:, 128 * j : 128 * (j + 1)], uh_sl[j])

        # ---- relu in halves on two engines (ACT + DVE run in parallel);
        # b1 is already accumulated in psum via the ones-row matmul.
        y2T = pool.tile([128, TC], F16, tag="y2T")
        nc.scalar.activation(
            y2T[:, 0:256], psumY[:, 0:256],
            mybir.ActivationFunctionType.Relu)
        nc.vector.tensor_scalar_max(y2T[:, 256:512], psumY[:, 256:512], 0.0)

        # ---- heads, task-major: one 128-task tile at a time --------------
        # cols of psumS[:, i, :]: d3, d5, d4, d6  (W3, W5, W4, W6 order)
        psumS = psum.tile([128, NTILE, 4], F32, tag="psumS")
        for i in range(NTILE):
            nc.tensor.matmul(
                psumS[:, i, :],
                lhsT=y2T[:, 128 * i : 128 * (i + 1)],
                rhs=upA[:, 2 * TC : 2 * TC + 4],
                start=True,
                stop=True,
            )

        # ---- combine ------------------------------------------------------
        # bh' folds -FAILC into the b6 column, so y6s = a4 + a6 - FAILC and
        # out = pv*y6s + FAILC.
        av = pool.tile([128, NTILE, 4], F32, tag="av")
        nc.vector.tensor_mul(av, psumS, mesc_s)
        nc.vector.tensor_add(
            av, av, misc_s[:, 4:8].unsqueeze(1).broadcast_to([128, NTILE, 4])
        )
        y6s = pool.tile([128, NTILE, 1], F32, tag="y6s")
        nc.vector.tensor_add(y6s, av[:, :, 2:3], av[:, :, 3:4])

        sg = pool.tile([128, NTILE, 2], F32, tag="sg")
        nc.scalar.activation(sg, av[:, :, 0:2], mybir.ActivationFunctionType.Sigmoid)

        pv = pool.tile([128, NTILE, 1], F32, tag="pv")
        nc.vector.tensor_mul(pv, sg[:, :, 0:1], sg[:, :, 1:2])
        om = pool.tile([128, NTILE, 1], F32, tag="om")
        nc.vector.tensor_mul(om, pv, y6s)
        outv = pool.tile([128, NTILE, 1], F32, tag="outv")
        nc.vector.tensor_scalar_add(outv, om, FAILC)

        nc.sync.dma_start(out=out[:, :], in_=outv[:, :, 0])

    return _split_sync_waits(nc)


def _split_sync_waits(nc, max_waits=1):
    """This container's walrus rejects >1 sem-wait per instruction
    ("Too many sync wait commands"); hoist extras onto same-engine NOPs."""
    nid = 0
    for f in nc.m.functions:
        for bb in f.blocks:
            new = []
            for inst in bb.instructions:
                si = inst.sync_info
                if si is None:
                    new.append(inst)
                    continue
                waits = list(si.on_wait or [])
                if len(waits) > max_waits:
                    for w in waits[:-max_waits]:
                        nop = mybir.InstNoOp(name=f"WSPL-{nid}", ins=[], outs=[])
                        nid += 1
                        nop.engine = inst.engine
                        nop.sync_info = mybir.SyncInfo(on_wait=[w], on_update=[])
                        new.append(nop)
                    inst.sync_info = mybir.SyncInfo(
                        on_wait=waits[-max_waits:], on_update=list(si.on_update or [])
                    )
                new.append(inst)
            bb.instructions = new
    return nc


_CACHED_NC = None


def _get_nc():
    global _CACHED_NC
    if _CACHED_NC is None:
        _CACHED_NC = _build_module()
    return _CACHED_NC


def _make_in_maps(inputs: dict) -> list[dict[str, np.ndarray]]:
    f32 = np.float32
    f16 = np.float16

    bb = np.asarray(inputs["backbone_y"], f32).reshape(T, D_BB)
    res = np.asarray(inputs["y_res"], f32).reshape(T, 5)
    fr = np.asarray(inputs["y_fr"], f32).reshape(T, 5)
    estep = np.asarray(inputs["y_estep"], f32).reshape(T, 6)
    enode = np.asarray(inputs["y_enode"], f32).reshape(T, E_N)
    ccl = np.asarray(inputs["y_ccluster"], f32).reshape(T, C_C)
    cnd = np.asarray(inputs["y_cnode"], f32).reshape(T, C_N)

    # outer3 features [T, 150] and per-task scalars (host precompute)
    o3 = np.einsum("tn,tm,to->tnmo", res, fr, estep).reshape(T, N_OUT)
    me = enode.mean(axis=1).astype(f32)
    sc = (ccl.sum(axis=1) * cnd.sum(axis=1)).astype(f32)

    # W1 packed fp16 with permuted rows: [bb (768) ; outer3 (150)]
    w1 = np.ascontiguousarray(np.asarray(inputs["W1"], f32))
    w1a = w1[0:N_OUT]        # outer3 rows
    w1b = w1[N_OUT:]         # bb rows [768, 128]
    w1f_c = np.ascontiguousarray(
        np.concatenate(
            [
                w1b.astype(f16).reshape(NBB, 128, D_H).transpose(1, 0, 2)
                .reshape(128, NBB * D_H),
                w1a[0:128].astype(f16),
            ],
            axis=1,
        )
    )  # [128, 896]
    b1_col = np.asarray(inputs["b1"], f32).reshape(D_H, 1)

    w3 = np.asarray(inputs["W3"], f32).reshape(D_H, 1)
    w4 = np.asarray(inputs["W4"], f32).reshape(D_H, 1)
    w5 = np.asarray(inputs["W5"], f32).reshape(D_H, 1)
    w6 = np.asarray(inputs["W6"], f32).reshape(D_H, 1)
    bh_row = np.array(
        [
            float(np.asarray(inputs["b3"]).reshape(-1)[0]),
            float(np.asarray(inputs["b5"]).reshape(-1)[0]),
            float(np.asarray(inputs["b4"]).reshape(-1)[0]),
            float(np.asarray(inputs["b6"]).reshape(-1)[0]) - FAILC,
        ],
        f32,
    )
    whf_c = np.concatenate([w3, w5, w4, w6], axis=1).astype(f16)  # [128, 4]
    # [128, 9]: wh | bh' | b1
    misc9_c = np.ascontiguousarray(
        np.concatenate(
            [
                np.concatenate([w3, w5, w4, w6], axis=1),
                np.broadcast_to(bh_row, (128, 4)),
                b1_col,
            ],
            axis=1,
        )
    )

    in_maps = []
    for c in range(NCORES):
        sl = slice(c * TC, (c + 1) * TC)
        uh_c = bb[sl].T.astype(f16)          # [768, TC], C-contiguous
        o3T = o3[sl].T.astype(f16)           # [150, TC]
        # packs in arrival order: o3a, uh0..5
        chunks = [o3T[0:128]] + [uh_c[128 * j : 128 * (j + 1)] for j in range(NBB)]
        st = np.stack(chunks, axis=1)        # [128, 7, TC]
        o3t_c = np.ascontiguousarray(
            np.concatenate(
                [
                    w1a[128:N_OUT].astype(f16),
                    o3T[128:N_OUT],
                    np.broadcast_to(b1_col.reshape(1, D_H), (22, D_H)).astype(f16),
                ],
                axis=1,
            )
        )  # [22, 768]
        mesc_c = np.ascontiguousarray(
            np.stack(
                [me[sl].reshape(NTILE, 128).T, sc[sl].reshape(NTILE, 128).T] * 2,
                axis=-1,
            )
        )  # [128, NTILE, 4] = me, sc, me, sc
        in_maps.append(
            {
                "w1f": w1f_c,
                "upkA": np.ascontiguousarray(
                    np.concatenate([st[:, 0], st[:, 1], whf_c], axis=1)
                ),
                "upkB": np.ascontiguousarray(st[:, 2:4]),
                "upkC": np.ascontiguousarray(st[:, 4:7]),
                "o3t": o3t_c,
                "mesc4": mesc_c,
                "misc9": misc9_c,
            }
        )
    return in_maps


def _assemble(results: list[dict[str, np.ndarray]]) -> np.ndarray:
    parts = [np.asarray(results[c]["out"]).T.reshape(-1) for c in range(NCORES)]
    return np.concatenate(parts)[None, :].astype(np.float32)


def _run(inputs: dict, trace: bool = False):
    nc = _get_nc()
    in_maps = _make_in_maps(inputs)
    kres = run_bass_kernel_spmd(
        nc, in_maps, core_ids=list(range(NCORES)), trace=trace
    )
    return _assemble(kres.results), kres


def kernel(**inputs) -> np.ndarray:
    out, _ = _run(inputs)
    return out
